# revision 18
# baseline (speedup 1.0000x reference)
"""nn_CausalSelfAttention (B=4, T=2048, C=768, H=12) on 8 Trainium2 cores.

The reference performs a *raw* reshape (B,3,T,H,hd) -> (3, B*H, T, hd) after
the qkv projection, which scrambles batch/qkv/head/token: attention unit
n in [0,48) takes q/k/v from flat 131072-element windows of the transposed
qkv buffer:
    q_n = buf[0*48+n], k_n = buf[1*48+n], v_n = buf[2*48+n]
where buf[u'] is window u' of the (B*3, T*768) row-major "block" sequence.
Window algebra: unit n's q lives in block u=(n//12) (a (batch, qkv-third)
pair), window w=n%12 covering block rows [1024*(w>=6) ..); six windows per
block half.

Sharding: core c in [0,8) handles units n in [6c, 6c+6) -> output batch
b_out=c//2, output heads 6*(c%2)..+6. Each core computes the three
half-blocks it needs (q/k/v sources, different batches), bounces them
through DRAM, re-reads its six windows (XBAR DMA-transpose for q/k, strided
window loads for v), runs causal attention per unit, and emits a partial
c_proj in bf16. Host sums the two partials per batch and adds b_proj.
"""

import sys

for _p in ("/opt/trn_rl_repo",):
    if _p not in sys.path:
        sys.path.insert(0, _p)

import numpy as np

import concourse.bacc as bacc
import concourse.bass as bass
import concourse.mybir as mybir
import concourse.tile as tile
from concourse import library_config
from concourse import masks

B, T, C, H, HD = 4, 2048, 768, 12, 64
HL = H // 2          # units per core
FQ = HL * HD         # 384 local output feature width
QC = 256             # query chunk
KP = 128             # key tile (partition dim)
GK = 6               # k-tiles per exp group
N_CORES = 8
WSZ = T * HD         # 131072, flat elems per window
HB = T // 2 * C      # 786432, flat elems per half block

FP32 = mybir.dt.float32
BF16 = mybir.dt.bfloat16
AX = mybir.AluOpType


def dma_transpose_win(eng, out, in_):
    """XBAR DMA transpose with 64-wide source tiles: [r, 64] -> [64, r].

    Mirrors BassEngine.dma_start_transpose but passes tile_src_cols=64
    (verified on HW), avoiding the 128-col minimum that would force a
    parity-scrambled [r/2, 128] view."""
    nc = eng.bass
    out2 = out.opt(keep_dims=frozenset((0, len(out.shape) - 1)))
    in2 = in_.opt(keep_dims=frozenset({0}))
    return eng.add_instruction(
        mybir.InstDmaTransposeAnt(
            name=f"I-{nc.next_id()}",
            ins=eng.lower_ap_dma(in2, for_isa=True),
            outs=eng.lower_ap_dma(out2, for_isa=True),
            tile_src_rows=nc.XBAR_TILE_SRC_ROWS,
            tile_src_cols=64,
        ))


def build_nc(t=T, debug=False, dump=False, reps=1):
    """Per-core program (identical on all 8 cores). `t` scales the attention
    sequence length for small-scale testing; window size scales with t.
    `reps` repeats the whole computation device-side (for timing)."""
    assert t % (2 * QC) == 0
    assert not (dump and reps > 1)
    n_qc = t // QC
    n_tt = t // 128
    n_cc = C // 128
    wsz = t * HD                 # window elems
    hb = (t // 2) * C            # half-block elems
    assert 6 * wsz == hb

    nc = bacc.Bacc("TRN2", target_bir_lowering=False, debug=debug,
                   num_devices=N_CORES)

    xT3_d = nc.dram_tensor("xT3", [C, 3 * (t // 2)], BF16, kind="ExternalInput")
    w3_d = nc.dram_tensor("w3", [C, 3 * C], BF16, kind="ExternalInput")
    b3_d = nc.dram_tensor("b3", [1, 3 * C], FP32, kind="ExternalInput")
    wp_d = nc.dram_tensor("wp", [FQ, C], BF16, kind="ExternalInput")
    out_d = nc.dram_tensor("out", [t, C], BF16, kind="ExternalOutput")

    if dump:
        qkT_dump = nc.dram_tensor("qkT_dump", [2, HL, 64, t], BF16,
                                  kind="ExternalOutput")
        vt_dump = nc.dram_tensor("vt_dump", [HL, 128, n_tt * (HD + 1)], BF16,
                                 kind="ExternalOutput")
        yT_dump = nc.dram_tensor("yT_dump", [FQ // 128, 128, t], BF16,
                                 kind="ExternalOutput")

    with tile.TileContext(nc) as tc:
        nc.gpsimd.load_library(library_config.attn)
        with (
            tc.tile_pool(name="const", bufs=1) as cp,
            tc.tile_pool(name="dramp", bufs=1, space="DRAM") as dp,
            # SBUF pools live at top level: scoping them inside the phases
            # would recycle their zones across phases, and the resulting
            # WAR hazards make phase-2 instructions wait on the tail of
            # phase-1's DMA queues.
            tc.tile_pool(name="ph1sb", bufs=4) as b_pool,
            tc.tile_pool(name="psb", bufs=4) as p_pool,
            tc.tile_pool(name="nrm", bufs=4) as n_pool,
            tc.tile_pool(name="osb", bufs=3) as o_pool,
        ):
            blk_d = dp.tile([3, hb], BF16, tag="blk")
            xT3 = cp.tile([128, n_cc * 3 * (t // 2)], BF16, tag="xT3")
            w3 = cp.tile([128, n_cc * 3 * C], BF16, tag="w3")
            b3r = cp.tile([1, 3 * C], FP32, tag="b3r")
            b3b = cp.tile([128, 3 * C], FP32, tag="b3b")
            # one [128, t] column block per window; the XBAR transpose
            # writes garbage on partitions 64:128 of its block (HW writes a
            # full 128-partition swath), so data lives in partitions 0:64
            # and the upper half of each block is sacrificial.
            qT = cp.tile([128, HL * t], BF16, tag="qT")
            kT = cp.tile([128, HL * t], BF16, tag="kT")
            vt = cp.tile([128, HL * n_tt * (HD + 1)], BF16, tag="vt")
            yT = cp.tile([128, (FQ // 128) * t], BF16, tag="yT")
            wp = cp.tile([128, (FQ // 128) * C], BF16, tag="wp")

            tb = 3 * (t // 2)
            for kc in range(n_cc):
                nc.sync.dma_start(out=xT3[:, tb * kc:tb * (kc + 1)],
                                  in_=xT3_d[128 * kc:128 * (kc + 1), :])
                nc.sync.dma_start(out=w3[:, 3 * C * kc:3 * C * (kc + 1)],
                                  in_=w3_d[128 * kc:128 * (kc + 1), :])
            for pc in range(FQ // 128):
                nc.sync.dma_start(out=wp[:, C * pc:C * (pc + 1)],
                                  in_=wp_d[128 * pc:128 * (pc + 1), :])
            nc.sync.dma_start(out=b3r[:], in_=b3_d[:])
            nc.gpsimd.partition_broadcast(b3b[:], b3r[0:1, :])
            b3k = cp.tile([128, C], BF16, tag="b3k")
            nc.vector.tensor_copy(b3k[:], b3b[:, C:2 * C])

            def xT3_c(kc, j):
                # x^T chunk kc for block j: [128, t//2]
                base = tb * kc + (t // 2) * j
                return xT3[:, base:base + t // 2]

            def w3_c(kc, j):
                return w3[:, 3 * C * kc + C * j:3 * C * kc + C * (j + 1)]

            def vt_u(wl):
                blk = n_tt * (HD + 1)
                return vt[:, blk * wl:blk * (wl + 1)]

            def yT_c(pc):
                return yT[:, t * pc:t * (pc + 1)]

            def wp_c(pc):
                return wp[:, C * pc:C * (pc + 1)]

            for _rep in range(reps):
                # ------------- Phase 1: three half-blocks -> DRAM -------------
                # Window ops (v loads, q/k XBAR transposes) are emitted
                # interleaved as soon as the rows they cover are written, so
                # the in-order SP sequencer doesn't head-of-line-block them.
                rows_per_win = wsz // C  # t*64/768 = t/12 rows per window

                def win_ready(w, ti_done):
                    # window w fully covered once (w+1)*wsz <= ti_done rows*C
                    return (w + 1) * wsz <= 128 * ti_done * C

                with tc.tile_pool(name="ph1ps", bufs=3,
                                  space="PSUM") as b_psum:
                    emitted = 0  # windows fully emitted (per-window v+qk ops)

                    def emit_windows(ti_done):
                        nonlocal emitted
                        while emitted < HL and win_ready(emitted, ti_done):
                            wl = emitted
                            win = slice(wsz * wl, wsz * (wl + 1))
                            v3 = vt_u(wl).rearrange("p (n e) -> p n e",
                                                    e=HD + 1)
                            nc.sync.dma_start(
                                out=v3[:, :, 0:HD],
                                in_=blk_d[2][win].rearrange(
                                    "(n p d) -> p n d", p=128, d=HD))
                            nc.vector.memset(v3[:, :, HD:HD + 1], 1.0)
                            for src, dst in ((0, qT), (1, kT)):
                                dma_transpose_win(
                                    nc.sync,
                                    dst[0:64, t * wl:t * (wl + 1)],
                                    blk_d[src][win].rearrange(
                                        "(r c) -> r c", c=HD))
                            emitted += 1

                    for ti in range(t // 2 // 128):
                        ob = b_pool.tile([128, 3 * C], BF16, tag="bsb")
                        for j in range(3):
                            ps = b_psum.tile([128, 1024], FP32, tag="bps")
                            for kc in range(n_cc):
                                for n0, n1 in ((0, 512), (512, C)):
                                    nc.tensor.matmul(
                                        ps[:, n0:n1],
                                        lhsT=xT3_c(kc, j)[:, 128 * ti:128 * (ti + 1)],
                                        rhs=w3_c(kc, j)[:, n0:n1],
                                        start=(kc == 0), stop=(kc == n_cc - 1),
                                    )
                            obj = ob[:, C * j:C * (j + 1)]
                            if j == 1:
                                # k third: PSUM->SBUF copy on the idle Act
                                # engine, then bias-add SBUF->SBUF on Pool
                                # (Pool cannot read PSUM).
                                nc.scalar.activation(
                                    obj, ps[:, :C],
                                    mybir.ActivationFunctionType.Copy)
                                nc.gpsimd.tensor_tensor(
                                    out=obj, in0=obj,
                                    in1=b3k[:], op=AX.add)
                            else:
                                nc.vector.scalar_tensor_tensor(
                                    out=obj, in0=ps[:, :C], scalar=0.0,
                                    in1=b3b[:, C * j:C * (j + 1)],
                                    op0=AX.add, op1=AX.add)
                        # one DMA scatters all three thirds to their blocks
                        ofs = 128 * ti * C
                        nc.sync.dma_start(
                            out=blk_d[:, ofs:ofs + 128 * C].rearrange(
                                "j (p f) -> p j f", f=C),
                            in_=ob[:].rearrange("p (j f) -> p j f", f=C))
                        emit_windows(ti + 1)
                    emit_windows(t // 2 // 128)
                    assert emitted == HL

                if dump:
                    for mc in range(HL):
                        nc.sync.dma_start(out=qkT_dump[0][mc],
                                          in_=qT[0:64, t * mc:t * (mc + 1)])
                        nc.sync.dma_start(out=qkT_dump[1][mc],
                                          in_=kT[0:64, t * mc:t * (mc + 1)])
                    for wl in range(HL):
                        nc.sync.dma_start(out=vt_dump[wl], in_=vt_u(wl))

                # ------------- Phase 2: causal attention per unit -------------
                # Flatten each unit's (chunk j, key-tile kt) pairs into one
                # slot sequence; exp/PSUM groups of GK slots span chunk
                # boundaries so every Act call is full-width. Within a chunk
                # the two diagonal (masked) tiles go first so their
                # exp->mask->av chain hides behind later slots' scores.
                with (
                    tc.tile_pool(name="sps", bufs=2, space="PSUM") as s_psum,
                    tc.tile_pool(name="yps", bufs=2, space="PSUM") as y_psum,
                ):
                    slots = []  # (wl, j, kt, first, last)
                    for wl in range(HL):
                        for j in range(n_qc):
                            nkt = (QC * (j + 1)) // KP
                            order = [2 * j, 2 * j + 1] + list(range(0, 2 * j))
                            assert len(order) == nkt
                            for i, kt in enumerate(order):
                                slots.append((wl, j, kt, i == 0, i == nkt - 1))

                    y_tiles = {}
                    for g0 in range(0, len(slots), GK):
                        grp = slots[g0:g0 + GK]
                        gsz = len(grp)
                        s_ps = s_psum.tile([128, GK * QC], FP32, tag="sps")
                        p_sb = p_pool.tile([128, GK * QC], BF16, tag="psb")
                        for tt, (wl, j, kt, _, _) in enumerate(grp):
                            st = t * wl
                            nc.tensor.matmul(
                                s_ps[:, QC * tt:QC * (tt + 1)],
                                lhsT=kT[0:64,
                                        st + KP * kt:st + KP * (kt + 1)],
                                rhs=qT[0:64,
                                       st + QC * j:st + QC * (j + 1)],
                                start=True, stop=True,
                            )
                        nc.scalar.activation(
                            p_sb[:, :QC * gsz], s_ps[:, :QC * gsz],
                            mybir.ActivationFunctionType.Exp)
                        for tt, (wl, j, kt, _, _) in enumerate(grp):
                            d = kt - (QC * j) // KP
                            if d >= 0:
                                # keep iff q_local - k_local - 128d >= 0
                                nc.gpsimd.affine_select(
                                    out=p_sb[:, QC * tt:QC * (tt + 1)],
                                    in_=p_sb[:, QC * tt:QC * (tt + 1)],
                                    pattern=[[1, QC]],
                                    channel_multiplier=-1,
                                    base=-KP * d,
                                    compare_op=AX.is_ge,
                                    fill=0.0,
                                )
                        for tt, (wl, j, kt, first, last) in enumerate(grp):
                            if first:
                                y_tiles[(wl, j)] = y_psum.tile(
                                    [65, QC], FP32, name="y_ps", tag="yps")
                            nc.tensor.matmul(
                                y_tiles[(wl, j)][:],
                                lhsT=vt_u(wl)[:, (HD + 1) * kt:
                                              (HD + 1) * kt + HD + 1],
                                rhs=p_sb[:, QC * tt:QC * (tt + 1)],
                                start=first, stop=last,
                            )
                            if last:
                                y_ps = y_tiles.pop((wl, j))
                                po = 64 * (wl % 2)
                                yT_h = yT_c(wl // 2)[po:po + 64, :]
                                rd = n_pool.tile([1, QC], FP32, tag="rd")
                                nc.vector.reciprocal(rd[0:1, :],
                                                     y_ps[64:65, :])
                                bc = n_pool.tile([64, QC], FP32, tag="bc")
                                nc.gpsimd.partition_broadcast(bc[:],
                                                              rd[0:1, :])
                                nc.vector.tensor_tensor(
                                    out=yT_h[:, QC * j:QC * (j + 1)],
                                    in0=y_ps[0:64, :], in1=bc[:],
                                    op=AX.mult)

                if dump:
                    for pc in range(FQ // 128):
                        nc.sync.dma_start(out=yT_dump[pc], in_=yT_c(pc))

                # ------------- Phase 3: output projection (partial) -----------
                with tc.tile_pool(name="ops", bufs=2,
                                  space="PSUM") as o_psum:
                    for ti in range(n_tt):
                        ps = o_psum.tile([128, 1024], FP32, tag="ops")
                        for pc in range(FQ // 128):
                            for n0, n1 in ((0, 512), (512, C)):
                                nc.tensor.matmul(
                                    ps[:, n0:n1],
                                    lhsT=yT_c(pc)[:, 128 * ti:128 * (ti + 1)],
                                    rhs=wp_c(pc)[:, n0:n1],
                                    start=(pc == 0), stop=(pc == FQ // 128 - 1),
                                )
                        ob = o_pool.tile([128, C], BF16, tag="osb")
                        if ti % 2 == 0:
                            nc.scalar.activation(
                                ob[:], ps[:, :C],
                                mybir.ActivationFunctionType.Copy)
                        else:
                            nc.vector.tensor_copy(ob[:], ps[:, :C])
                        nc.sync.dma_start(
                            out=out_d[128 * ti:128 * (ti + 1), :], in_=ob[:])

    nc.compile()
    return nc


def shard_inputs(x, W_attn, b_attn, W_proj, t=T):
    """Host-side shard + cast. Returns in_maps (one dict per core)."""
    scale = np.float32(1.0 / np.sqrt(HD))
    bf16 = np.dtype(mybir.dt.np(BF16))
    Wpr = W_proj.reshape(H, HD, C)
    in_maps = []
    for c in range(N_CORES):
        c2, half = c // 2, c % 2
        rows = slice((t // 2) * half, (t // 2) * (half + 1))
        xs, ws, bs = [], [], []
        for j, u in enumerate([c2, 4 + c2, 8 + c2]):
            b_j, s_j = divmod(u, 3)
            sc = scale if j == 0 else np.float32(1.0)
            xs.append(np.ascontiguousarray(x[b_j, rows].T))
            ws.append(W_attn[:, C * s_j:C * (s_j + 1)] * sc)
            bs.append(b_attn[C * s_j:C * (s_j + 1)] * sc)
        xT3 = np.concatenate(xs, axis=1).astype(bf16)          # [C, 3*t/2]
        w3 = np.concatenate(ws, axis=1).astype(bf16)           # [C, 3C]
        b3 = np.concatenate(bs).reshape(1, 3 * C).astype(np.float32)
        hs = slice(HL * half, HL * (half + 1))
        wp = np.ascontiguousarray(Wpr[hs].reshape(FQ, C)).astype(bf16)
        in_maps.append({"xT3": np.ascontiguousarray(xT3), "w3": w3,
                        "b3": b3, "wp": wp})
    return in_maps


LAST_RESULTS = None


def kernel(x, W_attn, b_attn, W_proj, b_proj):
    global LAST_RESULTS
    from concourse.bass_utils import run_bass_kernel_spmd

    x = np.asarray(x, dtype=np.float32)
    W_attn = np.asarray(W_attn, dtype=np.float32)
    b_attn = np.asarray(b_attn, dtype=np.float32)
    W_proj = np.asarray(W_proj, dtype=np.float32)
    b_proj = np.asarray(b_proj, dtype=np.float32)

    nc = build_nc()
    in_maps = shard_inputs(x, W_attn, b_attn, W_proj)
    res = run_bass_kernel_spmd(nc, in_maps, list(range(N_CORES)))
    LAST_RESULTS = res

    out = np.empty((B, T, C), dtype=np.float32)
    for b in range(B):
        out[b] = res.results[2 * b]["out"].astype(np.float32) \
            + res.results[2 * b + 1]["out"].astype(np.float32) \
            + b_proj[None, :]
    return out
